# revision 3
# baseline (speedup 1.0000x reference)
"""DEQ block (Anderson acceleration, 6 iters, m=3) on 8 trn2 NeuronCores.

Data-parallel over batch: each core gets 512 of 4096 samples; W_z/W_x/b
replicated.  Per core the 512 samples are processed as two sequential
halves of 256 (2 m-tiles of 128) so all per-sample state stays SBUF
resident in fp32.  Matmuls run as float32r (FP22 reads, fp32 accumulate).

Per iteration i (z update, sample-major state):
  f   = tanh(z @ W_z + xwx)            PE (+identity-matmul xwx add) + ACT
  g   = f - z                          DVE scalar_tensor_tensor, in place
  u   = beta*g + z                     DVE scalar_tensor_tensor
  i<3:  z' = u  (buffer alias, no copy)
  i>=3: 2x2 regularized Anderson solve from 3 fresh dots
        P=<g,g> (ACT square+accum), Q1=<g,g1>, Q2=<g,g2> (DVE TTR),
        gram history terms reused from previous iterations' P/Q1;
        z' = s0*u + gamma1*u1 + gamma2*u2  (ACT scale + 2 DVE STT)
"""

import sys

sys.path.insert(0, "/opt/trn_rl_repo")

import numpy as np
from contextlib import ExitStack

import concourse.bass as bass
import concourse.tile as tile
from concourse import bacc, mybir, masks
from concourse import bass_utils

F32 = mybir.dt.float32
F32R = mybir.dt.float32r
F16 = mybir.dt.float16
ALU = mybir.AluOpType
ACTF = mybir.ActivationFunctionType

B, D = 4096, 2048
NCORES = 8
BC = B // NCORES          # 512 samples per core
NHALF = 2                 # sequential halves per core
CH = BC // NHALF          # 256 samples per half
MT = CH // 128            # 2 m-tiles per half
KT = D // 128             # 16 k-tiles
NT = D // 512             # 4 n-slices
RWZ = 4                   # W_z k-tiles kept SBUF resident; rest streamed
MAX_ITER, MAND = 6, 3
BETA, LAM = 0.8, 1e-4

_CACHE = {}

import os
NITER = int(os.environ.get("K_NITER", str(MAX_ITER)))   # iterations per half
NHALVES = int(os.environ.get("K_NHALVES", "2"))
FAKE_RES = int(os.environ.get("K_FAKE_RESIDENT", "0"))  # timing expt: no W stream


def _r(ap):
    return ap.bitcast(F32R)


def _build():
    nc = bacc.Bacc("TRN2", target_bir_lowering=False, debug=False,
                   num_devices=NCORES)

    x_d = nc.dram_tensor("x", [BC, D], F32, kind="ExternalInput").ap()
    wz_d = nc.dram_tensor("W_z", [D, D], F32, kind="ExternalInput").ap()
    wx_d = nc.dram_tensor("W_x", [D, D], F32, kind="ExternalInput").ap()
    b_d = nc.dram_tensor("b", [D], F32, kind="ExternalInput").ap()
    out_d = nc.dram_tensor("z_out", [BC, D], F32, kind="ExternalOutput").ap()
    # staging for half-1's xwx (computed in phase 0, reloaded at half 1)
    xwx1_d = nc.dram_tensor("xwx1_stage", [MT, 128, D], F16, kind="Internal").ap()

    with tile.TileContext(nc) as tc, ExitStack() as ctx:
        # ---------------- pools ----------------
        state = ctx.enter_context(tc.tile_pool(name="state", bufs=1))

        def persist(shape, nm):
            return state.tile(shape, F32, tag=nm, name=nm)

        wz16 = [state.tile([128, D], F16, tag=f"wz16_{k}", name=f"wz16_{k}")
                for k in range(KT)]
        zbuf = [persist([128, D], f"zbuf{m}") for m in range(MT)]
        gsl = [[persist([128, D], f"g{j}_{m}") for m in range(MT)]
               for j in range(3)]
        usl = [[persist([128, D], f"u{j}_{m}") for m in range(MT)]
               for j in range(3)]
        xwx = [state.tile([128, D], F16, tag=f"xwx{m}", name=f"xwx{m}")
               for m in range(MT)]
        ident = persist([128, 128], "ident")

        wpool = ctx.enter_context(tc.tile_pool(name="wstream", bufs=2))
        ztpool = ctx.enter_context(tc.tile_pool(name="ztp", bufs=33))
        dots = ctx.enter_context(tc.tile_pool(name="dots", bufs=40))
        typs = ctx.enter_context(tc.tile_pool(name="tpsum", bufs=3, space="PSUM"))
        yps = ctx.enter_context(tc.tile_pool(name="ypsum", bufs=4, space="PSUM"))

        pdump = state.tile([128, 512], F32, tag="pdump", name="pdump")
        qdump = state.tile([128, 512], F32, tag="qdump", name="qdump")
        masks.make_identity(nc, ident[:])
        identh = state.tile([128, 128], F16, tag="identh", name="identh")
        nc.vector.tensor_copy(identh[:], ident[:])
        rid = ident[:]          # fp32, rhs of fp32 transposes
        ridh = identh[:]        # fp16, lhsT of the xwx identity-matmul

        # W_z: DMA fp32 rows in, round to resident fp16 tiles on DVE
        for k in range(KT):
            for j in range(2):
                wrow = wpool.tile([128, 1024], F32, tag="w", name=f"wl{k}_{j}")
                nc.sync.dma_start(wrow[:], wz_d[k * 128:(k + 1) * 128,
                                               j * 1024:(j + 1) * 1024])
                nc.vector.tensor_copy(wz16[k][:, j * 1024:(j + 1) * 1024],
                                      wrow[:])

        def uw(j, ap):
            # usl[0]/usl[1] memlocs are fp32r-consumed (XT backing): every
            # engine write into them must round to fp32r for the verifier
            return _r(ap) if j in (0, 1) else ap

        def stt(out, in0, scalar, in1, op0, op1):
            nc.vector.scalar_tensor_tensor(
                out=out, in0=in0, scalar=scalar, in1=in1, op0=op0, op1=op1)

        # XT backing: 16 transposed-x k-rows [128, 512] live inside the
        # (not yet used) u-ring tiles during phase 0.
        def xt_sl(k, q):
            back = [usl[0][0], usl[0][1], usl[1][0], usl[1][1]][k // 4]
            off = (k % 4) * 512 + q * 128
            return back[:, off:off + 128]

        # ---------------- phase 0: xwx for all 4 quarter-tiles ----------------
        for q in range(4):
            xs = []
            for h2 in range(2):
                xst = wpool.tile([128, 1024], F32, tag="w", name=f"xst{q}_{h2}")
                nc.sync.dma_start(xst[:], x_d[q * 128:(q + 1) * 128,
                                               h2 * 1024:(h2 + 1) * 1024])
                xs.append(xst)
            for k in range(KT):
                tp = typs.tile([128, 128], F32, tag="tp", name=f"xtp{q}_{k}")
                src = xs[k // 8][:, (k % 8) * 128:(k % 8 + 1) * 128]
                nc.tensor.transpose(tp[:], src, rid)
                nc.scalar.copy(_r(xt_sl(k, q)), tp[:])


        b2d = b_d.rearrange("(p n) -> p n", p=1)
        for n in range(NT):
            b1 = wpool.tile([1, 512], F32, tag="w", name=f"b1_{n}")
            nc.sync.dma_start(b1[:], b2d[:, n * 512:(n + 1) * 512])
            bsl = wpool.tile([128, 512], F32, tag="w", name=f"bsl{n}")
            nc.gpsimd.partition_broadcast(bsl[:], b1[:])
            ps = [yps.tile([128, 512], F32, tag="yp", name=f"xwps{n}_{q}") for q in range(4)]
            for k in range(KT):
                wt = wpool.tile([128, 512], F32R, tag="w", name=f"wx{n}_{k}")
                nc.sync.dma_start(wt[:], _r(wx_d[k * 128:(k + 1) * 128,
                                                 n * 512:(n + 1) * 512]))
                for q in range(4):
                    nc.tensor.matmul(ps[q][:], _r(xt_sl(k, q)), wt[:],
                                     start=(k == 0), stop=(k == KT - 1))
            for q in range(4):
                if q < MT:
                    dst = xwx[q][:, n * 512:(n + 1) * 512]
                else:
                    dst = zbuf[q - MT].bitcast(F16)[:, n * 512:(n + 1) * 512]
                stt(dst, ps[q][:], 1.0, bsl[:], ALU.mult, ALU.add)
        for m in range(MT):
            nc.sync.dma_start(xwx1_d[m],
                              zbuf[m].bitcast(F16)[:, 0:D])

        # ---------------- per-half iterations ----------------
        def emit_half(h):
            if h == 1:
                for m in range(MT):
                    nc.sync.dma_start(xwx[m][:], xwx1_d[m])

            hist = {}  # (kind, i, m) -> [128,1] ap

            # iteration 0: z=0 -> g0 = tanh(xwx), u0 = beta*g0, z1 aliases u0
            for m in range(MT):
                nc.scalar.activation(gsl[0][m][:], xwx[m][:], ACTF.Tanh)
                nc.vector.tensor_scalar_mul(_r(usl[0][m][:]), gsl[0][m][:], BETA)

            for i in range(1, NITER):
                gi, ui = gsl[i % 3], usl[i % 3]
                g1, g2 = gsl[(i - 1) % 3], gsl[(i - 2) % 3]
                u1, u2 = usl[(i - 1) % 3], usl[(i - 2) % 3]
                zc = usl[i - 1] if i <= 3 else zbuf  # current z (alias)

                # transpose z into lhsT k-tiles
                zt = {}
                for m in range(MT):
                    for k in range(KT):
                        tp = typs.tile([128, 128], F32, tag="tp", name=f"tp{h}_{i}_{m}_{k}")
                        nc.tensor.transpose(
                            tp[:], zc[m][:, k * 128:(k + 1) * 128], rid)
                        zs = ztpool.tile([128, 128], F16, tag="zt",
                                         name=f"zt{h}_{i}_{m}_{k}")
                        nc.scalar.copy(zs[:], tp[:])
                        zt[m, k] = zs

                # matmul + xwx add + tanh, n-slice major
                for n in range(NT):
                    ps = [yps.tile([128, 512], F32, tag="yp", name=f"yp{h}_{i}_{n}_{m}")
                          for m in range(MT)]
                    for k in range(KT):
                        wsl = wz16[k][:, n * 512:(n + 1) * 512]
                        for m in range(MT):
                            nc.tensor.matmul(ps[m][:], zt[m, k][:], wsl,
                                             start=(k == 0), stop=False)
                    for m in range(MT):
                        nc.tensor.matmul(ps[m][:], ridh,
                                         xwx[m][:, n * 512:(n + 1) * 512],
                                         start=False, stop=True)
                        nc.scalar.activation(gi[m][:, n * 512:(n + 1) * 512],
                                             ps[m][:], ACTF.Tanh)

                for m in range(MT):
                    # g = f - z ; u = beta*g + z
                    stt(gi[m][:], gi[m][:], 1.0, zc[m][:], ALU.mult, ALU.subtract)
                    stt(uw(i % 3, ui[m][:]), gi[m][:], BETA, zc[m][:], ALU.mult, ALU.add)

                    # P = <g,g> on ACT (square + accum), dumped to PSUM
                    pc = dots.tile([128, 4], F32, tag="d", name=f"pc{h}_{i}_{m}")
                    for c in range(4):
                        nc.scalar.activation(pdump[:],
                                             gi[m][:, c * 512:(c + 1) * 512],
                                             ACTF.Square,
                                             accum_out=pc[:, c:c + 1])
                    pp = dots.tile([128, 1], F32, tag="d", name=f"p{h}_{i}_{m}")
                    nc.vector.tensor_reduce(pp[:], pc[:], mybir.AxisListType.X,
                                            ALU.add)
                    hist["P", i, m] = pp

                    def ttr_dot(gh, nm):
                        qc = dots.tile([128, 4], F32, tag="d", name=f"{nm}c")
                        for c in range(4):
                            nc.vector.scalar_tensor_tensor(
                                out=qdump[:],
                                in0=gi[m][:, c * 512:(c + 1) * 512],
                                scalar=1.0,
                                in1=gh[m][:, c * 512:(c + 1) * 512],
                                op0=ALU.mult, op1=ALU.mult,
                                accum_out=qc[:, c:c + 1])
                        qq = dots.tile([128, 1], F32, tag="d", name=nm)
                        nc.vector.tensor_reduce(qq[:], qc[:],
                                                mybir.AxisListType.X, ALU.add)
                        return qq

                    if i >= 2:
                        hist["Q1", i, m] = ttr_dot(g1, f"q1_{h}_{i}_{m}")
                    if i >= 3:
                        q2t = ttr_dot(g2, f"q2_{h}_{i}_{m}")

                        P = hist["P", i, m][:]
                        Q1 = hist["Q1", i, m][:]
                        Q2 = q2t[:]
                        S11 = hist["P", i - 1, m][:]
                        S12 = hist["Q1", i - 1, m][:]
                        S22 = hist["P", i - 2, m][:]

                        def tnew(nm):
                            return dots.tile([128, 1], F32, tag="d",
                                             name=f"{nm}_{h}_{i}_{m}")[:]

                        def ts(out, in0, s1, s2, op0, op1=None):
                            nc.vector.tensor_scalar(out, in0, s1, s2, op0,
                                                    *( [op1] if op1 else []))

                        def aff(out, in_, scale, bias):
                            nc.scalar.activation(out, in_, ACTF.Identity,
                                                 bias=bias, scale=scale)

                        r0 = tnew("r0"); ts(r0, P, Q1, None, ALU.subtract)
                        r1 = tnew("r1"); ts(r1, P, Q2, None, ALU.subtract)
                        a1 = tnew("a1"); aff(a1, Q1, -2.0, S11)
                        av = tnew("av"); ts(av, a1, LAM, P, ALU.add, ALU.add)
                        d1 = tnew("d1"); aff(d1, Q2, -2.0, S22)
                        dv = tnew("dv"); ts(dv, d1, LAM, P, ALU.add, ALU.add)
                        b1 = tnew("b1"); aff(b1, Q2, -1.0, S12)
                        bv = tnew("bv"); ts(bv, b1, r0, None, ALU.add)
                        t4 = tnew("t4"); aff(t4, av, dv, 0.0)
                        t5 = tnew("t5"); nc.scalar.square(t5, bv)
                        det = tnew("det")
                        ts(det, t4, 1e-8, t5, ALU.add, ALU.subtract)
                        idet = tnew("idet"); nc.vector.reciprocal(idet, det)
                        g1a = tnew("g1a"); aff(g1a, dv, r0, 0.0)
                        g1b = tnew("g1b"); ts(g1b, bv, r1, None, ALU.mult)
                        g1c = tnew("g1c"); ts(g1c, g1a, g1b, None, ALU.subtract)
                        gam1 = tnew("gam1"); ts(gam1, g1c, idet, None, ALU.mult)
                        g2a = tnew("g2a"); aff(g2a, av, r1, 0.0)
                        g2b = tnew("g2b"); ts(g2b, bv, r0, None, ALU.mult)
                        g2c = tnew("g2c"); ts(g2c, g2a, g2b, None, ALU.subtract)
                        gam2 = tnew("gam2"); ts(gam2, g2c, idet, None, ALU.mult)
                        s0a = tnew("s0a")
                        ts(s0a, gam1, -1.0, gam2, ALU.mult, ALU.subtract)
                        s0 = tnew("s0"); aff(s0, s0a, 1.0, 1.0)

                        # z' = s0*u + gam1*u1 + gam2*u2 (u2 slot is scratch)
                        ju = (i - 2) % 3
                        nc.scalar.mul(uw(ju, u2[m][:]), u2[m][:], gam2)
                        stt(uw(ju, u2[m][:]), u1[m][:], gam1, u2[m][:],
                            ALU.mult, ALU.add)
                        stt(zbuf[m][:], ui[m][:], s0, u2[m][:],
                            ALU.mult, ALU.add)

            for m in range(MT):
                q = h * MT + m
                nc.sync.dma_start(out_d[q * 128:(q + 1) * 128, :], zbuf[m][:])

        emit_half(0)
        if NHALVES > 1:
            emit_half(1)

    nc.compile()
    return nc


def kernel(x_input, W_z, W_x, b):
    x_input = np.ascontiguousarray(x_input, dtype=np.float32)
    W_z = np.ascontiguousarray(W_z, dtype=np.float32)
    W_x = np.ascontiguousarray(W_x, dtype=np.float32)
    b = np.ascontiguousarray(b, dtype=np.float32)

    if "nc" not in _CACHE:
        _CACHE["nc"] = _build()
    nc = _CACHE["nc"]

    in_maps = [{
        "x": x_input[i * BC:(i + 1) * BC],
        "W_z": W_z, "W_x": W_x, "b": b,
    } for i in range(NCORES)]

    res = bass_utils.run_bass_kernel_spmd(nc, in_maps,
                                          core_ids=list(range(NCORES)))
    out = np.concatenate([res.results[i]["z_out"] for i in range(NCORES)],
                         axis=0)
    return out.astype(np.float32)


def profile_run(x_input, W_z, W_x, b):
    """Profiled run (NTFF trace) returning HW exec time in ns."""
    x_input = np.ascontiguousarray(x_input, dtype=np.float32)
    W_z = np.ascontiguousarray(W_z, dtype=np.float32)
    W_x = np.ascontiguousarray(W_x, dtype=np.float32)
    b = np.ascontiguousarray(b, dtype=np.float32)

    if "nc" not in _CACHE:
        _CACHE["nc"] = _build()
    nc = _CACHE["nc"]

    in_maps = [{
        "x": x_input[i * BC:(i + 1) * BC],
        "W_z": W_z, "W_x": W_x, "b": b,
    } for i in range(NCORES)]

    import glob as globmod
    import tempfile

    from trn_agent_boot.trn_boot import _ntff_profile_via_ctypes
    from concourse import bass2jax
    from concourse._compat import FishPath
    import gauge.profiler

    hook = _ntff_profile_via_ctypes("/opt/axon/libaxon_pjrt.so")
    if hook is None:
        print("no NTFF profile hook available")
        return None
    neff_dir = tempfile.mkdtemp(prefix="bass_prof_")
    with hook(neff_dir, [0]):
        bass2jax.run_bass_via_pjrt(nc, in_maps, n_cores=NCORES)
    ntffs = globmod.glob(os.path.join(neff_dir, "*_body*.ntff"))
    if not ntffs:
        print("no ntff files found:", sorted(os.listdir(neff_dir)))
        return None
    profile = gauge.profiler.Profile(
        profile_path=FishPath(neff_dir),
        kernel_dev_mode=True,
        profile_on_exit=False,
        bass_kernel=nc.m,
        offline_processing=True,
        fname="*_body*",
    )
    pr = profile.to_perfetto(model_index=(0,))
    if not pr:
        print("to_perfetto produced no results")
        return None
    print(f"trace: {pr[0].trace_path}")
    print(f"profile artifacts in {neff_dir}")
    return pr[0].exec_time_ns



# revision 13
# speedup vs baseline: 1.9288x; 1.9288x over previous
"""DEQ block (Anderson acceleration, 6 iters, m=3) on 8 trn2 NeuronCores.

Data-parallel over batch: each core gets 512 of 4096 samples; W_z/W_x/b
replicated.  All 512 samples (4 m-tiles of 128) stay SBUF resident in
fp16, so the pointwise/dot/solve work of one n-slice overlaps the PE
matmuls of the next.  W_z is cast to fp16 in DRAM once (SWDGE cast-DMA)
and streamed per iteration in (n,k) consumption order through a small
SBUF ring; x/W_x stream through a deep dual-queue (SP+ACT HWDGE) ring.

Per iteration i (z update, sample-major state):
  f   = tanh(z @ W_z + xwx)            PE (+identity-matmul xwx add) + ACT
  g   = f - z ; u = beta*g + z         DVE STT, fp16
  i<3:  z' = u  (buffer alias, no copy)
  i>=3: 2x2 regularized Anderson solve from 3 fresh dots
        P=<g,g> (ACT square+accum), Q1=<g,g1>, Q2=<g,g2> (DVE STT),
        gram history terms reused from previous iterations' P/Q1;
        z' = s0*u + gamma1*u1 + gamma2*u2  (ACT scale + 2 DVE STT)
"""

import os
import sys

sys.path.insert(0, "/opt/trn_rl_repo")

import numpy as np
from contextlib import ExitStack

import concourse.bass as bass
import concourse.tile as tile
from concourse import bacc, mybir, masks
from concourse import bass_utils

F32 = mybir.dt.float32
F16 = mybir.dt.float16
ALU = mybir.AluOpType
ACTF = mybir.ActivationFunctionType

B, D = 4096, 2048
NCORES = 8
BC = B // NCORES          # 512 samples per core
MT = BC // 128            # 4 m-tiles
KT = D // 128             # 16 k-tiles
NT = D // 512             # 4 n-slices
MAX_ITER, MAND = 6, 3
BETA, LAM = 0.8, 1e-4

_CACHE = {}

NITER = int(os.environ.get("K_NITER", str(MAX_ITER)))
WZRING = int(os.environ.get("K_WZRING", "12"))


def _build():
    nc = bacc.Bacc("TRN2", target_bir_lowering=False, debug=False,
                   num_devices=NCORES)

    x_d = nc.dram_tensor("x", [BC, D], F32, kind="ExternalInput").ap()
    wz_d = nc.dram_tensor("W_z", [D, D], F32, kind="ExternalInput").ap()
    wx_d = nc.dram_tensor("W_x", [D, D], F32, kind="ExternalInput").ap()
    b_d = nc.dram_tensor("b", [D], F32, kind="ExternalInput").ap()
    out_d = nc.dram_tensor("z_out", [BC, D], F32, kind="ExternalOutput").ap()
    # W_z staged fp16 in DRAM (cast once, streamed every iteration)
    wzst_d = nc.dram_tensor("wz_f16", [D, D], F16, kind="Internal").ap()

    with tile.TileContext(nc) as tc, ExitStack() as ctx:
        # ---------------- pools ----------------
        state = ctx.enter_context(tc.tile_pool(name="state", bufs=1))

        def persist(shape, nm, dt=F16):
            return state.tile(shape, dt, tag=nm, name=nm)

        zbuf = [persist([128, D], f"zbuf{m}") for m in range(MT)]
        gsl = [[persist([128, D], f"g{j}_{m}") for m in range(MT)]
               for j in range(3)]
        usl = [[persist([128, D], f"u{j}_{m}") for m in range(MT)]
               for j in range(3)]
        xwx = [persist([128, D], f"xwx{m}") for m in range(MT)]
        zt = [persist([128, D], f"zt{m}") for m in range(MT)]
        bsl = [persist([128, 512], f"bsl{n}", F32) for n in range(NT)]
        pdump = persist([128, 512], "pdump", F32)
        qdump = persist([128, 512], "qdump", F32)
        ident = persist([128, 128], "ident", F32)
        # P/Q1 dot history rings (per m-tile, 3 generations)
        Ph = [[persist([128, 1], f"P{j}_{m}", F32) for j in range(3)]
              for m in range(MT)]
        Q1h = [[persist([128, 1], f"Q1_{j}_{m}", F32) for j in range(3)]
               for m in range(MT)]

        wpool = ctx.enter_context(tc.tile_pool(name="wstream", bufs=8))
        wxf16 = ctx.enter_context(tc.tile_pool(name="wxf16", bufs=6))
        wzr = ctx.enter_context(tc.tile_pool(name="wzring", bufs=WZRING))
        dots = ctx.enter_context(tc.tile_pool(name="dots", bufs=128))
        typs = ctx.enter_context(tc.tile_pool(name="tpsum", bufs=2, space="PSUM"))
        yps = ctx.enter_context(tc.tile_pool(name="ypsum", bufs=6, space="PSUM"))

        masks.make_identity(nc, ident[:])
        identh = persist([128, 128], "identh")
        nc.vector.tensor_copy(identh[:], ident[:])
        rid = ident[:]          # fp32, rhs for fp32 transposes
        ridh = identh[:]        # fp16, lhsT of the xwx identity-matmul

        def stt(out, in0, scalar, in1, op0, op1, **kw):
            nc.vector.scalar_tensor_tensor(
                out=out, in0=in0, scalar=scalar, in1=in1, op0=op0, op1=op1,
                **kw)

        # W_z -> fp16 DRAM staging; 4 contiguous SWDGE cast-DMAs on the
        # Pool queue run concurrently with the HWDGE x/W_x stream.
        for kg in range(4):
            nc.gpsimd.dma_start(wzst_d[kg * 512:(kg + 1) * 512, :],
                                wz_d[kg * 512:(kg + 1) * 512, :])

        dmaq = [nc.sync, nc.scalar]   # two HWDGE queues, round-robin
        qi = 0

        def stream_dma(dst, src):
            nonlocal qi
            dmaq[qi % 2].dma_start(dst, src)
            qi += 1

        # xT backing: 16 transposed-x k-tiles [128, 512] fp16 live inside
        # the (not yet used) u0 tiles during phase 0.
        def xt_sl(k, q):
            return usl[0][k // 4][:, (k % 4) * 512 + q * 128:
                                  (k % 4) * 512 + (q + 1) * 128]

        # ---------------- phase 0: xwx for all 4 quarter-tiles ----------------
        for q in range(4):
            xs = []
            for h4 in range(4):
                xst = wpool.tile([128, 512], F32, tag="w", name=f"xst{q}_{h4}")
                stream_dma(xst[:], x_d[q * 128:(q + 1) * 128,
                                       h4 * 512:(h4 + 1) * 512])
                xs.append(xst)
            for kb in range(4):
                tp = typs.tile([128, 512], F32, tag="tp", name=f"xtp{q}_{kb}")
                for j in range(4):
                    k = kb * 4 + j
                    nc.tensor.transpose(
                        tp[:, j * 128:(j + 1) * 128],
                        xs[k // 4][:, (k % 4) * 128:(k % 4 + 1) * 128], rid)
                for j in range(4):
                    k = kb * 4 + j
                    nc.vector.tensor_copy(xt_sl(k, q),
                                          tp[:, j * 128:(j + 1) * 128])

        b2d = b_d.rearrange("(p n) -> p n", p=1)
        for n in range(NT):
            b1 = wpool.tile([1, 512], F32, tag="w", name=f"b1_{n}")
            nc.gpsimd.dma_start(b1[:], b2d[:, n * 512:(n + 1) * 512])
            nc.gpsimd.partition_broadcast(bsl[n][:], b1[:])

        for n in range(NT):
            ps = [yps.tile([128, 512], F32, tag="yp", name=f"xwps{n}_{q}")
                  for q in range(4)]
            for k in range(KT):
                wt32 = wpool.tile([128, 512], F32, tag="w", name=f"wx{n}_{k}")
                stream_dma(wt32[:], wx_d[k * 128:(k + 1) * 128,
                                         n * 512:(n + 1) * 512])
                wt = wxf16.tile([128, 512], F16, tag="wx16", name=f"wxh{n}_{k}")
                nc.vector.tensor_copy(wt[:], wt32[:])
                for q in range(4):
                    nc.tensor.matmul(ps[q][:], xt_sl(k, q), wt[:],
                                     start=(k == 0), stop=(k == KT - 1))
            for q in range(4):
                stt(xwx[q][:, n * 512:(n + 1) * 512], ps[q][:], 1.0,
                    bsl[n][:], ALU.mult, ALU.add)

        # ---------------- iterations ----------------
        hist_q2 = {}

        # iteration 0: z=0 -> g0 = tanh(xwx), u0 = beta*g0, z1 aliases u0
        for m in range(MT):
            nc.scalar.activation(gsl[0][m][:], xwx[m][:], ACTF.Tanh)
            nc.vector.tensor_scalar_mul(usl[0][m][:], gsl[0][m][:], BETA)

        for i in range(1, NITER):
            gi, ui = gsl[i % 3], usl[i % 3]
            g1, g2 = gsl[(i - 1) % 3], gsl[(i - 2) % 3]
            u1, u2 = usl[(i - 1) % 3], usl[(i - 2) % 3]
            zc = usl[i - 1] if i <= 3 else zbuf  # current z (alias)

            # transpose z into lhsT k-tiles (PE) + psum->sbuf fp16 (DVE)
            for m in range(MT):
                for kb in range(4):
                    tp = typs.tile([128, 512], F16, tag="tp",
                                   name=f"tp{i}_{m}_{kb}")
                    for j in range(4):
                        k = kb * 4 + j
                        nc.tensor.transpose(
                            tp[:, j * 128:(j + 1) * 128],
                            zc[m][:, k * 128:(k + 1) * 128], ridh)
                    nc.vector.tensor_copy(
                        zt[m][:, kb * 512:(kb + 1) * 512], tp[:])

            # W_z chunk stream for this iteration, (n,k) consumption order
            wchunk = {}
            for n in range(NT):
                for k in range(KT):
                    wt = wzr.tile([128, 512], F16, tag="wz",
                                  name=f"wz{i}_{n}_{k}")
                    nc.sync.dma_start(wt[:], wzst_d[k * 128:(k + 1) * 128,
                                                    n * 512:(n + 1) * 512])
                    wchunk[n, k] = wt

            # matmul + xwx add + tanh + pointwise + dot chunks, n-major
            pc = [dots.tile([128, 4], F32, tag="d", name=f"pc{i}_{m}")
                  for m in range(MT)]
            q1c = [dots.tile([128, 4], F32, tag="d", name=f"q1c{i}_{m}")
                   for m in range(MT)]
            q2c = [dots.tile([128, 4], F32, tag="d", name=f"q2c{i}_{m}")
                   for m in range(MT)]
            for n in range(NT):
                sl = slice(n * 512, (n + 1) * 512)
                ps = [yps.tile([128, 512], F32, tag="yp", name=f"yp{i}_{n}_{m}")
                      for m in range(MT)]
                for k in range(KT):
                    wsl = wchunk[n, k][:]
                    for m in range(MT):
                        nc.tensor.matmul(ps[m][:],
                                         zt[m][:, k * 128:(k + 1) * 128], wsl,
                                         start=(k == 0), stop=False)
                for m in range(MT):
                    nc.tensor.matmul(ps[m][:], ridh, xwx[m][:, sl],
                                     start=False, stop=True)
                for m in range(MT):
                    # f into the g slot (fp16), then g = f - z, u = b*g + z
                    nc.scalar.activation(gi[m][:, sl], ps[m][:], ACTF.Tanh)
                    stt(gi[m][:, sl], gi[m][:, sl], 1.0, zc[m][:, sl],
                        ALU.mult, ALU.subtract)
                    stt(ui[m][:, sl], gi[m][:, sl], BETA, zc[m][:, sl],
                        ALU.mult, ALU.add)
                    # dot chunks
                    nc.scalar.activation(pdump[:], gi[m][:, sl], ACTF.Square,
                                         accum_out=pc[m][:, n:n + 1])
                    if i >= 2:
                        stt(qdump[:], gi[m][:, sl], 1.0, g1[m][:, sl],
                            ALU.mult, ALU.mult,
                            accum_out=q1c[m][:, n:n + 1])
                    if i >= 3:
                        stt(qdump[:], gi[m][:, sl], 1.0, g2[m][:, sl],
                            ALU.mult, ALU.mult,
                            accum_out=q2c[m][:, n:n + 1])

            for m in range(MT):
                nc.vector.tensor_reduce(Ph[m][i % 3][:], pc[m][:],
                                        mybir.AxisListType.X, ALU.add)
                if i >= 2:
                    nc.vector.tensor_reduce(Q1h[m][i % 3][:], q1c[m][:],
                                            mybir.AxisListType.X, ALU.add)
                if i >= 3:
                    q2 = dots.tile([128, 1], F32, tag="d", name=f"q2_{i}_{m}")
                    nc.vector.tensor_reduce(q2[:], q2c[m][:],
                                            mybir.AxisListType.X, ALU.add)

                    P = Ph[m][i % 3][:]
                    Q1 = Q1h[m][i % 3][:]
                    Q2 = q2[:]
                    S11 = Ph[m][(i - 1) % 3][:]
                    S12 = Q1h[m][(i - 1) % 3][:]
                    S22 = Ph[m][(i - 2) % 3][:]

                    def tnew(nm):
                        return dots.tile([128, 1], F32, tag="d",
                                         name=f"{nm}_{i}_{m}")[:]

                    def ts(out, in0, s1, s2, op0, op1=None):
                        nc.vector.tensor_scalar(out, in0, s1, s2, op0,
                                                *([op1] if op1 else []))

                    # all-DVE 2x2 solve chain (no cross-engine hops)
                    r0 = tnew("r0"); ts(r0, P, Q1, None, ALU.subtract)
                    r1 = tnew("r1"); ts(r1, P, Q2, None, ALU.subtract)
                    a1 = tnew("a1"); stt(a1, Q1, -2.0, S11, ALU.mult, ALU.add)
                    av = tnew("av"); ts(av, a1, LAM, P, ALU.add, ALU.add)
                    d1 = tnew("d1"); stt(d1, Q2, -2.0, S22, ALU.mult, ALU.add)
                    dv = tnew("dv"); ts(dv, d1, LAM, P, ALU.add, ALU.add)
                    b1 = tnew("b1"); stt(b1, Q2, -1.0, S12, ALU.mult, ALU.add)
                    bv = tnew("bv"); ts(bv, b1, r0, None, ALU.add)
                    t4 = tnew("t4"); ts(t4, av, dv, None, ALU.mult)
                    t5 = tnew("t5"); stt(t5, bv, 1.0, bv, ALU.mult, ALU.mult)
                    det = tnew("det")
                    ts(det, t4, 1e-8, t5, ALU.add, ALU.subtract)
                    idet = tnew("idet"); nc.vector.reciprocal(idet, det)
                    g1a = tnew("g1a"); ts(g1a, dv, r0, None, ALU.mult)
                    g1b = tnew("g1b"); ts(g1b, bv, r1, None, ALU.mult)
                    g1c = tnew("g1c"); ts(g1c, g1a, g1b, None, ALU.subtract)
                    gam1 = tnew("gam1"); ts(gam1, g1c, idet, None, ALU.mult)
                    g2a = tnew("g2a"); ts(g2a, av, r1, None, ALU.mult)
                    g2b = tnew("g2b"); ts(g2b, bv, r0, None, ALU.mult)
                    g2c = tnew("g2c"); ts(g2c, g2a, g2b, None, ALU.subtract)
                    gam2 = tnew("gam2"); ts(gam2, g2c, idet, None, ALU.mult)
                    s0a = tnew("s0a")
                    ts(s0a, gam1, -1.0, gam2, ALU.mult, ALU.subtract)
                    s0 = tnew("s0"); ts(s0, s0a, 1.0, None, ALU.add)

                    # z' = s0*u + gam1*u1 + gam2*u2 (u2 slot is scratch)
                    nc.scalar.mul(u2[m][:], u2[m][:], gam2)
                    stt(u2[m][:], u1[m][:], gam1, u2[m][:],
                        ALU.mult, ALU.add)
                    stt(zbuf[m][:], ui[m][:], s0, u2[m][:],
                        ALU.mult, ALU.add)

                if i == NITER - 1:
                    zfin = zbuf[m] if (i >= 3 or NITER <= 3) else ui[m]
                    # fp16 -> fp32 cast on the SWDGE out-DMA
                    nc.gpsimd.dma_start(out_d[m * 128:(m + 1) * 128, :],
                                        zfin[:])

    nc.compile()
    return nc


def kernel(x_input, W_z, W_x, b):
    x_input = np.ascontiguousarray(x_input, dtype=np.float32)
    W_z = np.ascontiguousarray(W_z, dtype=np.float32)
    W_x = np.ascontiguousarray(W_x, dtype=np.float32)
    b = np.ascontiguousarray(b, dtype=np.float32)

    if "nc" not in _CACHE:
        _CACHE["nc"] = _build()
    nc = _CACHE["nc"]

    in_maps = [{
        "x": x_input[i * BC:(i + 1) * BC],
        "W_z": W_z, "W_x": W_x, "b": b,
    } for i in range(NCORES)]

    res = bass_utils.run_bass_kernel_spmd(nc, in_maps,
                                          core_ids=list(range(NCORES)))
    out = np.concatenate([res.results[i]["z_out"] for i in range(NCORES)],
                         axis=0)
    return out.astype(np.float32)


def profile_run(x_input, W_z, W_x, b):
    """Profiled run (NTFF trace) returning HW exec time in ns."""
    x_input = np.ascontiguousarray(x_input, dtype=np.float32)
    W_z = np.ascontiguousarray(W_z, dtype=np.float32)
    W_x = np.ascontiguousarray(W_x, dtype=np.float32)
    b = np.ascontiguousarray(b, dtype=np.float32)

    if "nc" not in _CACHE:
        _CACHE["nc"] = _build()
    nc = _CACHE["nc"]

    in_maps = [{
        "x": x_input[i * BC:(i + 1) * BC],
        "W_z": W_z, "W_x": W_x, "b": b,
    } for i in range(NCORES)]

    import glob as globmod
    import tempfile

    from trn_agent_boot.trn_boot import _ntff_profile_via_ctypes
    from concourse import bass2jax
    from concourse._compat import FishPath
    import gauge.profiler

    hook = _ntff_profile_via_ctypes("/opt/axon/libaxon_pjrt.so")
    if hook is None:
        print("no NTFF profile hook available")
        return None
    neff_dir = tempfile.mkdtemp(prefix="bass_prof_")
    with hook(neff_dir, [0]):
        bass2jax.run_bass_via_pjrt(nc, in_maps, n_cores=NCORES)
    ntffs = globmod.glob(os.path.join(neff_dir, "*_body*.ntff"))
    if not ntffs:
        print("no ntff files found:", sorted(os.listdir(neff_dir)))
        return None
    profile = gauge.profiler.Profile(
        profile_path=FishPath(neff_dir),
        kernel_dev_mode=True,
        profile_on_exit=False,
        bass_kernel=nc.m,
        offline_processing=True,
        fname="*_body*",
    )
    pr = profile.to_perfetto(model_index=(0,))
    if not pr:
        print("to_perfetto produced no results")
        return None
    print(f"trace: {pr[0].trace_path}")
    print(f"profile artifacts in {neff_dir}")
    return pr[0].exec_time_ns


# revision 21
# speedup vs baseline: 2.0052x; 1.0396x over previous
"""DEQ block (Anderson acceleration, 6 iters, m=3) on 8 trn2 NeuronCores.

Data-parallel over batch: each core gets 512 of 4096 samples; W_z/W_x/b
replicated.  All 512 samples (4 m-tiles of 128) stay SBUF resident in
fp16, so the pointwise/dot/solve work of one n-slice overlaps the PE
matmuls of the next.  W_z is cast to fp16 in DRAM once (SWDGE cast-DMA)
and streamed per iteration in (n,k) consumption order through a small
SBUF ring; x/W_x stream through a deep dual-queue (SP+ACT HWDGE) ring.

Per iteration i (z update, sample-major state):
  f   = tanh(z @ W_z + xwx)            PE (+identity-matmul xwx add) + ACT
  g   = f - z ; u = beta*g + z         DVE STT, fp16
  i<3:  z' = u  (buffer alias, no copy)
  i>=3: 2x2 regularized Anderson solve from 3 fresh dots
        P=<g,g> (ACT square+accum), Q1=<g,g1>, Q2=<g,g2> (DVE STT),
        gram history terms reused from previous iterations' P/Q1;
        z' = s0*u + gamma1*u1 + gamma2*u2  (ACT scale + 2 DVE STT)
"""

import os
import sys

sys.path.insert(0, "/opt/trn_rl_repo")

import numpy as np
from contextlib import ExitStack

import concourse.bass as bass
import concourse.tile as tile
from concourse import bacc, mybir, masks
from concourse import bass_utils

F32 = mybir.dt.float32
F16 = mybir.dt.float16
ALU = mybir.AluOpType
ACTF = mybir.ActivationFunctionType

B, D = 4096, 2048
NCORES = 8
BC = B // NCORES          # 512 samples per core
MT = BC // 128            # 4 m-tiles
KT = D // 128             # 16 k-tiles
NT = D // 512             # 4 n-slices
MAX_ITER, MAND = 6, 3
BETA, LAM = 0.8, 1e-4

_CACHE = {}

NITER = int(os.environ.get("K_NITER", str(MAX_ITER)))
WZRING = int(os.environ.get("K_WZRING", "12"))


def _build():
    nc = bacc.Bacc("TRN2", target_bir_lowering=False, debug=False,
                   num_devices=NCORES)

    x_d = nc.dram_tensor("x", [BC, D], F32, kind="ExternalInput").ap()
    wz_d = nc.dram_tensor("W_z", [D, D], F32, kind="ExternalInput").ap()
    wx_d = nc.dram_tensor("W_x", [D, D], F32, kind="ExternalInput").ap()
    b_d = nc.dram_tensor("b", [D], F32, kind="ExternalInput").ap()
    out_d = nc.dram_tensor("z_out", [BC, D], F32, kind="ExternalOutput").ap()
    # W_z staged fp16 in DRAM (cast once, streamed every iteration)
    wzst_d = nc.dram_tensor("wz_f16", [D, D], F16, kind="Internal").ap()

    with tile.TileContext(nc) as tc, ExitStack() as ctx:
        # ---------------- pools ----------------
        state = ctx.enter_context(tc.tile_pool(name="state", bufs=1))

        def persist(shape, nm, dt=F16):
            return state.tile(shape, dt, tag=nm, name=nm)

        zbuf = [persist([128, D], f"zbuf{m}") for m in range(MT)]
        gsl = [[persist([128, D], f"g{j}_{m}") for m in range(MT)]
               for j in range(3)]
        usl = [[persist([128, D], f"u{j}_{m}") for m in range(MT)]
               for j in range(3)]
        xwx = [persist([128, D], f"xwx{m}") for m in range(MT)]
        zt = [persist([128, D], f"zt{m}") for m in range(MT)]
        bsl = [persist([128, 512], f"bsl{n}", F32) for n in range(NT)]
        pdump = persist([128, 512], "pdump", F32)
        qdump = persist([128, 512], "qdump", F32)
        ident = persist([128, 128], "ident", F32)
        # P/Q1 dot history rings, m-batched [128, MT] (3 generations)
        Ph = [persist([128, MT], f"P{j}", F32) for j in range(3)]
        Q1h = [persist([128, MT], f"Q1_{j}", F32) for j in range(3)]

        wpool = ctx.enter_context(tc.tile_pool(name="wstream", bufs=8))
        wxf16 = ctx.enter_context(tc.tile_pool(name="wxf16", bufs=6))
        wzr = ctx.enter_context(tc.tile_pool(name="wzring", bufs=WZRING))
        dots = ctx.enter_context(tc.tile_pool(name="dots", bufs=128))
        typs = ctx.enter_context(tc.tile_pool(name="tpsum", bufs=2, space="PSUM"))
        yps = ctx.enter_context(tc.tile_pool(name="ypsum", bufs=6, space="PSUM"))

        masks.make_identity(nc, ident[:])
        identh = persist([128, 128], "identh")
        nc.vector.tensor_copy(identh[:], ident[:])
        rid = ident[:]          # fp32, rhs for fp32 transposes
        ridh = identh[:]        # fp16, lhsT of the xwx identity-matmul

        def stt(out, in0, scalar, in1, op0, op1, **kw):
            nc.vector.scalar_tensor_tensor(
                out=out, in0=in0, scalar=scalar, in1=in1, op0=op0, op1=op1,
                **kw)

        # W_z -> fp16 DRAM staging; 4 contiguous SWDGE cast-DMAs on the
        # Pool queue run concurrently with the HWDGE x/W_x stream.
        for kg in range(4):
            nc.gpsimd.dma_start(wzst_d[kg * 512:(kg + 1) * 512, :],
                                wz_d[kg * 512:(kg + 1) * 512, :])

        dmaq = [nc.sync, nc.scalar]   # two HWDGE queues, round-robin
        qi = 0

        def stream_dma(dst, src):
            nonlocal qi
            dmaq[qi % 2].dma_start(dst, src)
            qi += 1

        # xT backing: 16 transposed-x k-tiles [128, 512] fp16 live inside
        # the (not yet used) u0 tiles during phase 0.
        def xt_sl(k, q):
            return usl[0][k // 4][:, (k % 4) * 512 + q * 128:
                                  (k % 4) * 512 + (q + 1) * 128]

        # ---------------- phase 0: xwx for all 4 quarter-tiles ----------------
        for q in range(4):
            xs = []
            for h4 in range(4):
                xst = wpool.tile([128, 512], F32, tag="w", name=f"xst{q}_{h4}")
                stream_dma(xst[:], x_d[q * 128:(q + 1) * 128,
                                       h4 * 512:(h4 + 1) * 512])
                xs.append(xst)
            for kb in range(4):
                tp = typs.tile([128, 512], F32, tag="tp", name=f"xtp{q}_{kb}")
                for j in range(4):
                    k = kb * 4 + j
                    nc.tensor.transpose(
                        tp[:, j * 128:(j + 1) * 128],
                        xs[k // 4][:, (k % 4) * 128:(k % 4 + 1) * 128], rid)
                for j in range(4):
                    k = kb * 4 + j
                    nc.vector.tensor_copy(xt_sl(k, q),
                                          tp[:, j * 128:(j + 1) * 128])

        b2d = b_d.rearrange("(p n) -> p n", p=1)
        for n in range(NT):
            b1 = wpool.tile([1, 512], F32, tag="w", name=f"b1_{n}")
            nc.gpsimd.dma_start(b1[:], b2d[:, n * 512:(n + 1) * 512])
            nc.gpsimd.partition_broadcast(bsl[n][:], b1[:])

        for n in range(NT):
            ps = [yps.tile([128, 512], F32, tag="yp", name=f"xwps{n}_{q}")
                  for q in range(4)]
            for k in range(KT):
                wt32 = wpool.tile([128, 512], F32, tag="w", name=f"wx{n}_{k}")
                stream_dma(wt32[:], wx_d[k * 128:(k + 1) * 128,
                                         n * 512:(n + 1) * 512])
                wt = wxf16.tile([128, 512], F16, tag="wx16", name=f"wxh{n}_{k}")
                nc.vector.tensor_copy(wt[:], wt32[:])
                for q in range(4):
                    nc.tensor.matmul(ps[q][:], xt_sl(k, q), wt[:],
                                     start=(k == 0), stop=(k == KT - 1))
            for q in range(4):
                stt(xwx[q][:, n * 512:(n + 1) * 512], ps[q][:], 1.0,
                    bsl[n][:], ALU.mult, ALU.add)

        # ---------------- iterations ----------------
        hist_q2 = {}

        # iteration 0: z=0 -> g0 = tanh(xwx), u0 = beta*g0, z1 aliases u0
        for m in range(MT):
            nc.scalar.activation(gsl[0][m][:], xwx[m][:], ACTF.Tanh)
            nc.vector.tensor_scalar_mul(usl[0][m][:], gsl[0][m][:], BETA)

        for i in range(1, NITER):
            gi, ui = gsl[i % 3], usl[i % 3]
            g1, g2 = gsl[(i - 1) % 3], gsl[(i - 2) % 3]
            u1, u2 = usl[(i - 1) % 3], usl[(i - 2) % 3]
            zc = usl[i - 1] if i <= 3 else zbuf  # current z (alias)

            # transpose z into lhsT k-tiles (PE) + psum->sbuf fp16 (DVE)
            for m in range(MT):
                for kb in range(4):
                    tp = typs.tile([128, 512], F16, tag="tp",
                                   name=f"tp{i}_{m}_{kb}")
                    for j in range(4):
                        k = kb * 4 + j
                        nc.tensor.transpose(
                            tp[:, j * 128:(j + 1) * 128],
                            zc[m][:, k * 128:(k + 1) * 128], ridh)
                    nc.scalar.copy(
                        zt[m][:, kb * 512:(kb + 1) * 512], tp[:])

            # W_z chunk stream for this iteration, (n,k) consumption order
            wchunk = {}
            for n in range(NT):
                for k in range(KT):
                    wt = wzr.tile([128, 512], F16, tag="wz",
                                  name=f"wz{i}_{n}_{k}")
                    nc.sync.dma_start(wt[:], wzst_d[k * 128:(k + 1) * 128,
                                                    n * 512:(n + 1) * 512])
                    wchunk[n, k] = wt

            # matmul + xwx add + tanh + pointwise + dot chunks, n-major;
            # chunk dot accumulators are m-batched: column m*4+n
            pca = dots.tile([128, 16], F32, tag="d", name=f"pca{i}")
            q1a = dots.tile([128, 16], F32, tag="d", name=f"q1a{i}")
            q2a = dots.tile([128, 16], F32, tag="d", name=f"q2a{i}")
            for n in range(NT):
                sl = slice(n * 512, (n + 1) * 512)
                ps = [yps.tile([128, 512], F32, tag="yp", name=f"yp{i}_{n}_{m}")
                      for m in range(MT)]
                for k in range(KT):
                    wsl = wchunk[n, k][:]
                    for m in range(MT):
                        nc.tensor.matmul(ps[m][:],
                                         zt[m][:, k * 128:(k + 1) * 128], wsl,
                                         start=(k == 0), stop=False)
                for m in range(MT):
                    nc.tensor.matmul(ps[m][:], ridh, xwx[m][:, sl],
                                     start=False, stop=True)
                for m in range(MT):
                    c = m * 4 + n
                    # f into the g slot (fp16), then g = f - z, u = b*g + z
                    nc.scalar.activation(gi[m][:, sl], ps[m][:], ACTF.Tanh)
                    stt(gi[m][:, sl], gi[m][:, sl], 1.0, zc[m][:, sl],
                        ALU.mult, ALU.subtract)
                    stt(ui[m][:, sl], gi[m][:, sl], BETA, zc[m][:, sl],
                        ALU.mult, ALU.add)
                    # dot chunks
                    nc.scalar.activation(pdump[:], gi[m][:, sl], ACTF.Square,
                                         accum_out=pca[:, c:c + 1])
                    if i >= 2:
                        stt(qdump[:], gi[m][:, sl], 1.0, g1[m][:, sl],
                            ALU.mult, ALU.mult,
                            accum_out=q1a[:, c:c + 1])
                    if i >= 3:
                        stt(qdump[:], gi[m][:, sl], 1.0, g2[m][:, sl],
                            ALU.mult, ALU.mult,
                            accum_out=q2a[:, c:c + 1])

            # m-batched reduces + single [128, MT] solve chain (all DVE)
            def red(dst, src):
                nc.vector.tensor_reduce(
                    dst, src.rearrange("p (m n) -> p m n", n=4),
                    mybir.AxisListType.X, ALU.add)

            red(Ph[i % 3][:], pca[:])
            if i >= 2:
                red(Q1h[i % 3][:], q1a[:])
            if i >= 3:
                q2 = dots.tile([128, MT], F32, tag="d", name=f"q2_{i}")
                red(q2[:], q2a[:])

                P = Ph[i % 3][:]
                Q1 = Q1h[i % 3][:]
                Q2 = q2[:]
                S11 = Ph[(i - 1) % 3][:]
                S12 = Q1h[(i - 1) % 3][:]
                S22 = Ph[(i - 2) % 3][:]

                def tnew(nm):
                    return dots.tile([128, MT], F32, tag="d",
                                     name=f"{nm}_{i}")[:]

                def tt(out, a, b, op):
                    nc.vector.tensor_tensor(out, a, b, op)

                r0 = tnew("r0"); tt(r0, P, Q1, ALU.subtract)
                r1 = tnew("r1"); tt(r1, P, Q2, ALU.subtract)
                a1 = tnew("a1"); stt(a1, Q1, -2.0, S11, ALU.mult, ALU.add)
                av = tnew("av"); stt(av, a1, LAM, P, ALU.add, ALU.add)
                d1 = tnew("d1"); stt(d1, Q2, -2.0, S22, ALU.mult, ALU.add)
                dv = tnew("dv"); stt(dv, d1, LAM, P, ALU.add, ALU.add)
                b1 = tnew("b1"); stt(b1, Q2, -1.0, S12, ALU.mult, ALU.add)
                bv = tnew("bv"); tt(bv, b1, r0, ALU.add)
                t4 = tnew("t4"); tt(t4, av, dv, ALU.mult)
                t5 = tnew("t5"); tt(t5, bv, bv, ALU.mult)
                det = tnew("det")
                stt(det, t4, 1e-8, t5, ALU.add, ALU.subtract)
                idet = tnew("idet"); nc.vector.reciprocal(idet, det)
                g1a = tnew("g1a"); tt(g1a, dv, r0, ALU.mult)
                g1b = tnew("g1b"); tt(g1b, bv, r1, ALU.mult)
                g1c = tnew("g1c"); tt(g1c, g1a, g1b, ALU.subtract)
                gam1 = tnew("gam1"); tt(gam1, g1c, idet, ALU.mult)
                g2a = tnew("g2a"); tt(g2a, av, r1, ALU.mult)
                g2b = tnew("g2b"); tt(g2b, bv, r0, ALU.mult)
                g2c = tnew("g2c"); tt(g2c, g2a, g2b, ALU.subtract)
                gam2 = tnew("gam2"); tt(gam2, g2c, idet, ALU.mult)
                s0a = tnew("s0a")
                stt(s0a, gam1, -1.0, gam2, ALU.mult, ALU.subtract)
                s0 = tnew("s0")
                nc.vector.tensor_scalar(s0, s0a, 1.0, None, ALU.add)

                # z' = s0*u + gam1*u1 + gam2*u2 (u2 slot is scratch),
                # chunked by n so next iteration's transposes start after
                # the first 512-col chunk; ACT does the gam2 scale
                for m in range(MT):
                    mm = slice(m, m + 1)
                    for n in range(NT):
                        sl = slice(n * 512, (n + 1) * 512)
                        nc.scalar.mul(u2[m][:, sl], u2[m][:, sl],
                                      gam2[:, mm])
                        stt(u2[m][:, sl], u1[m][:, sl], gam1[:, mm],
                            u2[m][:, sl], ALU.mult, ALU.add)
                        stt(zbuf[m][:, sl], ui[m][:, sl], s0[:, mm],
                            u2[m][:, sl], ALU.mult, ALU.add)

            if i == NITER - 1:
                for m in range(MT):
                    zfin = zbuf[m] if (i >= 3 or NITER <= 3) else ui[m]
                    # fp16 -> fp32 cast on the SWDGE out-DMA
                    nc.gpsimd.dma_start(out_d[m * 128:(m + 1) * 128, :],
                                        zfin[:])

    nc.compile()
    return nc


def kernel(x_input, W_z, W_x, b):
    x_input = np.ascontiguousarray(x_input, dtype=np.float32)
    W_z = np.ascontiguousarray(W_z, dtype=np.float32)
    W_x = np.ascontiguousarray(W_x, dtype=np.float32)
    b = np.ascontiguousarray(b, dtype=np.float32)

    if "nc" not in _CACHE:
        _CACHE["nc"] = _build()
    nc = _CACHE["nc"]

    in_maps = [{
        "x": x_input[i * BC:(i + 1) * BC],
        "W_z": W_z, "W_x": W_x, "b": b,
    } for i in range(NCORES)]

    res = bass_utils.run_bass_kernel_spmd(nc, in_maps,
                                          core_ids=list(range(NCORES)))
    out = np.concatenate([res.results[i]["z_out"] for i in range(NCORES)],
                         axis=0)
    return out.astype(np.float32)


def profile_run(x_input, W_z, W_x, b):
    """Profiled run (NTFF trace) returning HW exec time in ns."""
    x_input = np.ascontiguousarray(x_input, dtype=np.float32)
    W_z = np.ascontiguousarray(W_z, dtype=np.float32)
    W_x = np.ascontiguousarray(W_x, dtype=np.float32)
    b = np.ascontiguousarray(b, dtype=np.float32)

    if "nc" not in _CACHE:
        _CACHE["nc"] = _build()
    nc = _CACHE["nc"]

    in_maps = [{
        "x": x_input[i * BC:(i + 1) * BC],
        "W_z": W_z, "W_x": W_x, "b": b,
    } for i in range(NCORES)]

    import glob as globmod
    import tempfile

    from trn_agent_boot.trn_boot import _ntff_profile_via_ctypes
    from concourse import bass2jax
    from concourse._compat import FishPath
    import gauge.profiler

    hook = _ntff_profile_via_ctypes("/opt/axon/libaxon_pjrt.so")
    if hook is None:
        print("no NTFF profile hook available")
        return None
    neff_dir = tempfile.mkdtemp(prefix="bass_prof_")
    with hook(neff_dir, [0]):
        bass2jax.run_bass_via_pjrt(nc, in_maps, n_cores=NCORES)
    ntffs = globmod.glob(os.path.join(neff_dir, "*_body*.ntff"))
    if not ntffs:
        print("no ntff files found:", sorted(os.listdir(neff_dir)))
        return None
    profile = gauge.profiler.Profile(
        profile_path=FishPath(neff_dir),
        kernel_dev_mode=True,
        profile_on_exit=False,
        bass_kernel=nc.m,
        offline_processing=True,
        fname="*_body*",
    )
    pr = profile.to_perfetto(model_index=(0,))
    if not pr:
        print("to_perfetto produced no results")
        return None
    print(f"trace: {pr[0].trace_path}")
    print(f"profile artifacts in {neff_dir}")
    return pr[0].exec_time_ns


# revision 27
# speedup vs baseline: 2.0380x; 1.0164x over previous
"""DEQ block (Anderson acceleration, 6 iters, m=3) on 8 trn2 NeuronCores.

Data-parallel over batch: each core gets 512 of 4096 samples; W_z/W_x/b
replicated.  All 512 samples (4 m-tiles of 128) stay SBUF resident in
fp16, so the pointwise/dot/solve work of one n-slice overlaps the PE
matmuls of the next.  W_z is cast to fp16 in DRAM once (SWDGE cast-DMA)
and streamed per iteration in (n,k) consumption order through a small
SBUF ring; x/W_x stream through a deep dual-queue (SP+ACT HWDGE) ring.

Per iteration i (z update, sample-major state):
  f   = tanh(z @ W_z + xwx)            PE (+identity-matmul xwx add) + ACT
  g   = f - z ; u = beta*g + z         DVE STT, fp16
  i<3:  z' = u  (buffer alias, no copy)
  i>=3: 2x2 regularized Anderson solve from 3 fresh dots
        P=<g,g> (ACT square+accum), Q1=<g,g1>, Q2=<g,g2> (DVE STT),
        gram history terms reused from previous iterations' P/Q1;
        z' = s0*u + gamma1*u1 + gamma2*u2  (ACT scale + 2 DVE STT)
"""

import os
import sys

sys.path.insert(0, "/opt/trn_rl_repo")

import numpy as np
from contextlib import ExitStack

import concourse.bass as bass
import concourse.tile as tile
from concourse import bacc, mybir, masks
from concourse import bass_utils

F32 = mybir.dt.float32
F16 = mybir.dt.float16
ALU = mybir.AluOpType
ACTF = mybir.ActivationFunctionType

B, D = 4096, 2048
NCORES = 8
BC = B // NCORES          # 512 samples per core
MT = BC // 128            # 4 m-tiles
KT = D // 128             # 16 k-tiles
NT = D // 512             # 4 n-slices
MAX_ITER, MAND = 6, 3
BETA, LAM = 0.8, 1e-4

_CACHE = {}

NITER = int(os.environ.get("K_NITER", str(MAX_ITER)))
WZRING = int(os.environ.get("K_WZRING", "12"))


def _build():
    nc = bacc.Bacc("TRN2", target_bir_lowering=False, debug=False,
                   num_devices=NCORES)

    x_d = nc.dram_tensor("x", [BC, D], F32, kind="ExternalInput").ap()
    wz_d = nc.dram_tensor("W_z", [D, D], F32, kind="ExternalInput").ap()
    wx_d = nc.dram_tensor("W_x", [D, D], F32, kind="ExternalInput").ap()
    b_d = nc.dram_tensor("b", [D], F32, kind="ExternalInput").ap()
    out_d = nc.dram_tensor("z_out", [BC, D], F32, kind="ExternalOutput").ap()
    # W_z staged fp16 in DRAM (cast once, streamed every iteration)
    wzst_d = nc.dram_tensor("wz_f16", [D, D], F16, kind="Internal").ap()

    with tile.TileContext(nc) as tc, ExitStack() as ctx:
        # ---------------- pools ----------------
        state = ctx.enter_context(tc.tile_pool(name="state", bufs=1))

        def persist(shape, nm, dt=F16):
            return state.tile(shape, dt, tag=nm, name=nm)

        zbuf = [persist([128, D], f"zbuf{m}") for m in range(MT)]
        gsl = [[persist([128, D], f"g{j}_{m}") for m in range(MT)]
               for j in range(3)]
        usl = [[persist([128, D], f"u{j}_{m}") for m in range(MT)]
               for j in range(3)]
        xwx = [persist([128, D], f"xwx{m}") for m in range(MT)]
        zt = [persist([128, D], f"zt{m}") for m in range(MT)]
        bsl = [persist([128, 512], f"bsl{n}", F32) for n in range(NT)]
        pdump = persist([128, 512], "pdump", F32)
        qdump = persist([128, 512], "qdump", F32)
        ident = persist([128, 128], "ident", F32)
        # P/Q1 dot history rings, m-batched [128, MT] (3 generations)
        Ph = [persist([128, MT], f"P{j}", F32) for j in range(3)]
        Q1h = [persist([128, MT], f"Q1_{j}", F32) for j in range(3)]

        wpool = ctx.enter_context(tc.tile_pool(name="wstream", bufs=10))
        wxf16 = ctx.enter_context(tc.tile_pool(name="wxf16", bufs=8))
        wzr = ctx.enter_context(tc.tile_pool(name="wzring", bufs=WZRING))
        dots = ctx.enter_context(tc.tile_pool(name="dots", bufs=128))
        typs = ctx.enter_context(tc.tile_pool(name="tpsum", bufs=3, space="PSUM"))
        yps = ctx.enter_context(tc.tile_pool(name="ypsum", bufs=5, space="PSUM"))

        masks.make_identity(nc, ident[:])
        identh = persist([128, 128], "identh")
        nc.vector.tensor_copy(identh[:], ident[:])
        rid = ident[:]          # fp32, rhs for fp32 transposes
        ridh = identh[:]        # fp16, lhsT of the xwx identity-matmul

        def stt(out, in0, scalar, in1, op0, op1, **kw):
            nc.vector.scalar_tensor_tensor(
                out=out, in0=in0, scalar=scalar, in1=in1, op0=op0, op1=op1,
                **kw)

        # b first (tiny, on the sync queue) so the xwx drains never wait
        b2d = b_d.rearrange("(p n) -> p n", p=1)
        for n in range(NT):
            b1 = wpool.tile([1, 512], F32, tag="w", name=f"b1_{n}")
            nc.sync.dma_start(b1[:], b2d[:, n * 512:(n + 1) * 512])
            nc.gpsimd.partition_broadcast(bsl[n][:], b1[:])

        # W_z -> fp16 DRAM staging; 4 contiguous SWDGE cast-DMAs on the
        # Pool queue run concurrently with the HWDGE x/W_x stream.
        for kg in range(4):
            nc.gpsimd.dma_start(wzst_d[kg * 512:(kg + 1) * 512, :],
                                wz_d[kg * 512:(kg + 1) * 512, :])

        dmaq = [nc.sync, nc.scalar]   # two HWDGE queues, round-robin
        qi = 0

        def stream_dma(dst, src):
            nonlocal qi
            dmaq[qi % 2].dma_start(dst, src)
            qi += 1

        # xT backing: 16 transposed-x k-tiles [128, 512] fp16 live inside
        # the (not yet used) u0 tiles during phase 0.
        def xt_sl(k, q):
            return usl[0][k // 4][:, (k % 4) * 512 + q * 128:
                                  (k % 4) * 512 + (q + 1) * 128]

        # ---------------- phase 0: xwx for all 4 quarter-tiles ----------------
        for q in range(4):
            xs = []
            for h4 in range(4):
                xst = wpool.tile([128, 512], F32, tag="w", name=f"xst{q}_{h4}")
                stream_dma(xst[:], x_d[q * 128:(q + 1) * 128,
                                       h4 * 512:(h4 + 1) * 512])
                xs.append(xst)
            for kb in range(4):
                tp = typs.tile([128, 512], F32, tag="tp", name=f"xtp{q}_{kb}")
                for j in range(4):
                    k = kb * 4 + j
                    nc.tensor.transpose(
                        tp[:, j * 128:(j + 1) * 128],
                        xs[k // 4][:, (k % 4) * 128:(k % 4 + 1) * 128], rid)
                for j in range(4):
                    k = kb * 4 + j
                    nc.vector.tensor_copy(xt_sl(k, q),
                                          tp[:, j * 128:(j + 1) * 128])

        for n in range(NT):
            ps = [yps.tile([128, 512], F32, tag="yp", name=f"xwps{n}_{q}")
                  for q in range(4)]
            for k in range(KT):
                wt32 = wpool.tile([128, 512], F32, tag="w", name=f"wx{n}_{k}")
                stream_dma(wt32[:], wx_d[k * 128:(k + 1) * 128,
                                         n * 512:(n + 1) * 512])
                wt = wxf16.tile([128, 512], F16, tag="wx16", name=f"wxh{n}_{k}")
                nc.vector.tensor_copy(wt[:], wt32[:])
                for q in range(4):
                    nc.tensor.matmul(ps[q][:], xt_sl(k, q), wt[:],
                                     start=(k == 0), stop=(k == KT - 1))
            for q in range(4):
                stt(xwx[q][:, n * 512:(n + 1) * 512], ps[q][:], 1.0,
                    bsl[n][:], ALU.mult, ALU.add)

        # ---------------- iterations ----------------
        hist_q2 = {}

        # iteration 0: z=0 -> g0 = tanh(xwx), u0 = beta*g0, z1 aliases u0
        for m in range(MT):
            nc.scalar.activation(gsl[0][m][:], xwx[m][:], ACTF.Tanh)
            nc.vector.tensor_scalar_mul(usl[0][m][:], gsl[0][m][:], BETA)

        for i in range(1, NITER):
            gi, ui = gsl[i % 3], usl[i % 3]
            g1, g2 = gsl[(i - 1) % 3], gsl[(i - 2) % 3]
            u1, u2 = usl[(i - 1) % 3], usl[(i - 2) % 3]
            zc = usl[i - 1] if i <= 3 else zbuf  # current z (alias)

            def emit_transposes(kb):
                # transpose z kb-block into lhsT k-tiles (PE) + psum->sbuf
                # fp16 (ACT); interleaved with the n=0 matmul sub-blocks so
                # PE restarts as soon as the first z' n-chunks land
                for m in range(MT):
                    tp = typs.tile([128, 512], F16, tag="tp",
                                   name=f"tp{i}_{m}_{kb}")
                    for j in range(4):
                        k = kb * 4 + j
                        nc.tensor.transpose(
                            tp[:, j * 128:(j + 1) * 128],
                            zc[m][:, k * 128:(k + 1) * 128], ridh)
                    nc.scalar.copy(
                        zt[m][:, kb * 512:(kb + 1) * 512], tp[:])

            # W_z chunk stream for this iteration, (n,k) consumption order
            wchunk = {}
            for n in range(NT):
                for k in range(KT):
                    wt = wzr.tile([128, 512], F16, tag="wz",
                                  name=f"wz{i}_{n}_{k}")
                    nc.sync.dma_start(wt[:], wzst_d[k * 128:(k + 1) * 128,
                                                    n * 512:(n + 1) * 512])
                    wchunk[n, k] = wt

            # matmul + xwx add + tanh + pointwise + dot chunks, n-major;
            # chunk dot accumulators are m-batched: column m*4+n
            pca = dots.tile([128, 16], F32, tag="d", name=f"pca{i}")
            q1a = dots.tile([128, 16], F32, tag="d", name=f"q1a{i}")
            q2a = dots.tile([128, 16], F32, tag="d", name=f"q2a{i}")
            for n in range(NT):
                sl = slice(n * 512, (n + 1) * 512)
                ps = [yps.tile([128, 512], F32, tag="yp", name=f"yp{i}_{n}_{m}")
                      for m in range(MT)]
                for k in range(KT):
                    if n == 0 and k % 4 == 0:
                        emit_transposes(k // 4)
                    wsl = wchunk[n, k][:]
                    for m in range(MT):
                        nc.tensor.matmul(ps[m][:],
                                         zt[m][:, k * 128:(k + 1) * 128], wsl,
                                         start=(k == 0), stop=False)
                for m in range(MT):
                    nc.tensor.matmul(ps[m][:], ridh, xwx[m][:, sl],
                                     start=False, stop=True)
                for m in range(MT):
                    c = m * 4 + n
                    # f into the g slot (fp16), then g = f - z, u = b*g + z
                    nc.scalar.activation(gi[m][:, sl], ps[m][:], ACTF.Tanh)
                    stt(gi[m][:, sl], gi[m][:, sl], 1.0, zc[m][:, sl],
                        ALU.mult, ALU.subtract)
                    stt(ui[m][:, sl], gi[m][:, sl], BETA, zc[m][:, sl],
                        ALU.mult, ALU.add)
                    # dot chunks
                    nc.scalar.activation(pdump[:], gi[m][:, sl], ACTF.Square,
                                         accum_out=pca[:, c:c + 1])
                    if i >= 2:
                        stt(qdump[:], gi[m][:, sl], 1.0, g1[m][:, sl],
                            ALU.mult, ALU.mult,
                            accum_out=q1a[:, c:c + 1])
                    if i >= 3:
                        stt(qdump[:], gi[m][:, sl], 1.0, g2[m][:, sl],
                            ALU.mult, ALU.mult,
                            accum_out=q2a[:, c:c + 1])

            # m-batched reduces + single [128, MT] solve chain (all DVE)
            def red(dst, src):
                nc.vector.tensor_reduce(
                    dst, src.rearrange("p (m n) -> p m n", n=4),
                    mybir.AxisListType.X, ALU.add)

            red(Ph[i % 3][:], pca[:])
            if i >= 2:
                red(Q1h[i % 3][:], q1a[:])
            if i >= 3:
                q2 = dots.tile([128, MT], F32, tag="d", name=f"q2_{i}")
                red(q2[:], q2a[:])

                P = Ph[i % 3][:]
                Q1 = Q1h[i % 3][:]
                Q2 = q2[:]
                S11 = Ph[(i - 1) % 3][:]
                S12 = Q1h[(i - 1) % 3][:]
                S22 = Ph[(i - 2) % 3][:]

                def tnew(nm):
                    return dots.tile([128, MT], F32, tag="d",
                                     name=f"{nm}_{i}")[:]

                def tt(out, a, b, op):
                    nc.vector.tensor_tensor(out, a, b, op)

                r0 = tnew("r0"); tt(r0, P, Q1, ALU.subtract)
                r1 = tnew("r1"); tt(r1, P, Q2, ALU.subtract)
                a1 = tnew("a1"); stt(a1, Q1, -2.0, S11, ALU.mult, ALU.add)
                av = tnew("av"); stt(av, a1, LAM, P, ALU.add, ALU.add)
                d1 = tnew("d1"); stt(d1, Q2, -2.0, S22, ALU.mult, ALU.add)
                dv = tnew("dv"); stt(dv, d1, LAM, P, ALU.add, ALU.add)
                b1 = tnew("b1"); stt(b1, Q2, -1.0, S12, ALU.mult, ALU.add)
                bv = tnew("bv"); tt(bv, b1, r0, ALU.add)
                t4 = tnew("t4"); tt(t4, av, dv, ALU.mult)
                t5 = tnew("t5"); tt(t5, bv, bv, ALU.mult)
                det = tnew("det")
                stt(det, t4, 1e-8, t5, ALU.add, ALU.subtract)
                idet = tnew("idet"); nc.vector.reciprocal(idet, det)
                g1a = tnew("g1a"); tt(g1a, dv, r0, ALU.mult)
                g1b = tnew("g1b"); tt(g1b, bv, r1, ALU.mult)
                g1c = tnew("g1c"); tt(g1c, g1a, g1b, ALU.subtract)
                gam1 = tnew("gam1"); tt(gam1, g1c, idet, ALU.mult)
                g2a = tnew("g2a"); tt(g2a, av, r1, ALU.mult)
                g2b = tnew("g2b"); tt(g2b, bv, r0, ALU.mult)
                g2c = tnew("g2c"); tt(g2c, g2a, g2b, ALU.subtract)
                gam2 = tnew("gam2"); tt(gam2, g2c, idet, ALU.mult)
                s0a = tnew("s0a")
                stt(s0a, gam1, -1.0, gam2, ALU.mult, ALU.subtract)
                s0 = tnew("s0")
                nc.vector.tensor_scalar(s0, s0a, 1.0, None, ALU.add)

                # z' = s0*u + gam1*u1 + gam2*u2 (u2 slot is scratch),
                # n-outer/m-inner chunks so z'(m3, n0) lands early and the
                # next iteration's kb0 transposes can start; ACT does the
                # gam2 scale
                for n in range(NT):
                    sl = slice(n * 512, (n + 1) * 512)
                    for m in range(MT):
                        mm = slice(m, m + 1)
                        nc.scalar.mul(u2[m][:, sl], u2[m][:, sl],
                                      gam2[:, mm])
                        stt(u2[m][:, sl], u1[m][:, sl], gam1[:, mm],
                            u2[m][:, sl], ALU.mult, ALU.add)
                        stt(zbuf[m][:, sl], ui[m][:, sl], s0[:, mm],
                            u2[m][:, sl], ALU.mult, ALU.add)
                        if i == NITER - 1:
                            # fp16 -> fp32 cast on the SWDGE out-DMA,
                            # chunk-interleaved to hide the tail
                            nc.gpsimd.dma_start(
                                out_d[m * 128:(m + 1) * 128, sl],
                                zbuf[m][:, sl])

            if i == NITER - 1 and i < 3:
                for m in range(MT):
                    nc.gpsimd.dma_start(out_d[m * 128:(m + 1) * 128, :],
                                        ui[m][:])

    nc.compile()
    return nc


def kernel(x_input, W_z, W_x, b):
    x_input = np.ascontiguousarray(x_input, dtype=np.float32)
    W_z = np.ascontiguousarray(W_z, dtype=np.float32)
    W_x = np.ascontiguousarray(W_x, dtype=np.float32)
    b = np.ascontiguousarray(b, dtype=np.float32)

    if "nc" not in _CACHE:
        _CACHE["nc"] = _build()
    nc = _CACHE["nc"]

    in_maps = [{
        "x": x_input[i * BC:(i + 1) * BC],
        "W_z": W_z, "W_x": W_x, "b": b,
    } for i in range(NCORES)]

    res = bass_utils.run_bass_kernel_spmd(nc, in_maps,
                                          core_ids=list(range(NCORES)))
    out = np.concatenate([res.results[i]["z_out"] for i in range(NCORES)],
                         axis=0)
    return out.astype(np.float32)


def profile_run(x_input, W_z, W_x, b):
    """Profiled run (NTFF trace) returning HW exec time in ns."""
    x_input = np.ascontiguousarray(x_input, dtype=np.float32)
    W_z = np.ascontiguousarray(W_z, dtype=np.float32)
    W_x = np.ascontiguousarray(W_x, dtype=np.float32)
    b = np.ascontiguousarray(b, dtype=np.float32)

    if "nc" not in _CACHE:
        _CACHE["nc"] = _build()
    nc = _CACHE["nc"]

    in_maps = [{
        "x": x_input[i * BC:(i + 1) * BC],
        "W_z": W_z, "W_x": W_x, "b": b,
    } for i in range(NCORES)]

    import glob as globmod
    import tempfile

    from trn_agent_boot.trn_boot import _ntff_profile_via_ctypes
    from concourse import bass2jax
    from concourse._compat import FishPath
    import gauge.profiler

    hook = _ntff_profile_via_ctypes("/opt/axon/libaxon_pjrt.so")
    if hook is None:
        print("no NTFF profile hook available")
        return None
    neff_dir = tempfile.mkdtemp(prefix="bass_prof_")
    with hook(neff_dir, [0]):
        bass2jax.run_bass_via_pjrt(nc, in_maps, n_cores=NCORES)
    ntffs = globmod.glob(os.path.join(neff_dir, "*_body*.ntff"))
    if not ntffs:
        print("no ntff files found:", sorted(os.listdir(neff_dir)))
        return None
    profile = gauge.profiler.Profile(
        profile_path=FishPath(neff_dir),
        kernel_dev_mode=True,
        profile_on_exit=False,
        bass_kernel=nc.m,
        offline_processing=True,
        fname="*_body*",
    )
    pr = profile.to_perfetto(model_index=(0,))
    if not pr:
        print("to_perfetto produced no results")
        return None
    print(f"trace: {pr[0].trace_path}")
    print(f"profile artifacts in {neff_dir}")
    return pr[0].exec_time_ns
